# revision 3
# baseline (speedup 1.0000x reference)
"""Multi-head causal attention (B=4, S=2048, D=512, H=8) on 8 TRN2 NeuronCores.

Sharding: core c -> (batch b = c//2, parity p = c%2). Each core computes full
K/V for its batch and attention + output projection for a strided half of the
q rows: 4 "slots" of 256 rows, slot j = global q rows
[256*(2j+p), 256*(2j+p)+256). Slot j attends to kv prefix [0, 512*(j+1)) —
identical shapes on every core (true SPMD, one program), causality inside the
slot handled by a host-prepared 0/1 transposed mask multiplied into the last
four kv tiles of each slot. No cross-core communication; host-side output
assembly is a pure concatenation of disjoint row blocks.

All matmuls run in float32r (full PE rate at moving-dim >= 256, ~2e-4 rel
err). Scores are computed transposed (S^T = K Q^T, kv on partitions) so the
softmax denominator comes free from the AV matmul via a ones-augmented V
(row 64 of the [65, 256] PSUM output = sum of exp), and no max-subtraction is
needed (scores are ~N(0,1); exp is safe in fp32, exp(masked)=0 exactly via
the multiplicative mask).
"""

import numpy as np

import concourse.mybir as mybir
import concourse.tile as tile
from concourse import bacc

F32 = mybir.dt.float32
F32R = mybir.dt.float32r
AF = mybir.ActivationFunctionType

B, S, D, H = 4, 2048, 512, 8
HD = D // H          # 64
P = 128
N_CORES = 8
QS = 256             # q rows per slot
SLOTS = 4
NQ = SLOTS * QS      # 1024 q rows per core
SCALE = float(1.0 / np.sqrt(HD))

_CACHE = {}


def _build():
    if "nc" in _CACHE:
        return _CACHE["nc"]
    nc = bacc.Bacc("TRN2", target_bir_lowering=False, debug=False,
                   num_devices=N_CORES)
    xT = nc.dram_tensor("xT", [D, S], F32R, kind="ExternalInput").ap()
    xqT = nc.dram_tensor("xqT", [D, NQ], F32R, kind="ExternalInput").ap()
    wq = nc.dram_tensor("wq", [D, D], F32R, kind="ExternalInput").ap()
    wk = nc.dram_tensor("wk", [D, D], F32R, kind="ExternalInput").ap()
    wv = nc.dram_tensor("wv", [D, D], F32R, kind="ExternalInput").ap()
    wo = nc.dram_tensor("wo", [D, D], F32R, kind="ExternalInput").ap()
    bqk_cols = nc.dram_tensor("bqk_cols", [P, 8], F32, kind="ExternalInput").ap()
    bv_row = nc.dram_tensor("bv_row", [1, D], F32R, kind="ExternalInput").ap()
    bo_row = nc.dram_tensor("bo_row", [1, D], F32R, kind="ExternalInput").ap()
    maskp = nc.dram_tensor("maskp", [P, SLOTS * 4 * QS], F32,
                           kind="ExternalInput").ap()
    out = nc.dram_tensor("out", [NQ, D], F32, kind="ExternalOutput").ap()

    with tile.TileContext(nc) as tc:
        _emit(nc, tc, xT, xqT, wq, wk, wv, wo, bqk_cols, bv_row, bo_row,
              maskp, out)
    nc.compile()
    _CACHE["nc"] = nc
    return nc


def _emit(nc, tc, xT, xqT, wq, wk, wv, wo, bqk_cols, bv_row, bo_row,
          maskp, out):
    from contextlib import ExitStack
    with ExitStack() as ctx:
        persist = ctx.enter_context(tc.tile_pool(name="persist", bufs=1))
        small = ctx.enter_context(tc.tile_pool(name="small", bufs=1))

        # persistent SBUF tensors
        QT = [persist.tile([P, NQ], F32R, tag=f"QT{ct}", name=f"QT{ct}") for ct in range(4)]
        KT = [persist.tile([P, S], F32R, tag=f"KT{ct}", name=f"KT{ct}") for ct in range(4)]
        V = persist.tile([P, (S // P) * 8 * (HD + 1)], F32R, tag="V")
        Vv = V[:].rearrange("p (t h c) -> p t h c", h=8, c=HD + 1)
        MK = persist.tile([P, SLOTS * 4 * QS], F32, tag="MK")
        VN = [persist.tile([P, NQ], F32R, tag=f"VN{dc}", name=f"VN{dc}") for dc in range(4)]
        WO = [persist.tile([P, D], F32R, tag=f"WO{dc}", name=f"WO{dc}") for dc in range(4)]

        bqkc = small.tile([P, 8], F32, tag="bqkc")
        nc.sync.dma_start(bqkc[:], bqk_cols[:])
        bvr = small.tile([1, D], F32R, tag="bvr")
        nc.sync.dma_start(bvr[:], bv_row[:])
        bor = small.tile([1, D], F32R, tag="bor")
        nc.sync.dma_start(bor[:], bo_row[:])
        ones_f = small.tile([1, D], F32, tag="ones_f")
        nc.vector.memset(ones_f[:], 1.0)
        ones_r = small.tile([1, D], F32R, tag="ones_r")
        nc.vector.tensor_copy(ones_r[:], ones_f[:])
        ones_blk = small.tile([P, (S // P) * 8], F32, tag="ones_blk")
        nc.vector.memset(ones_blk[:], 1.0)
        nc.sync.dma_start(MK[:], maskp[:])
        for dc in range(4):
            nc.sync.dma_start(WO[dc][:], wo[P * dc:P * (dc + 1), :])

        # ---- projections ----
        with (
            tc.tile_pool(name="xload", bufs=10) as xload,
            tc.tile_pool(name="wpool", bufs=1) as wpool,
            tc.tile_pool(name="psp", bufs=6, space="PSUM") as psp,
        ):
            WQ, WK, WV = [], [], []
            for name, src, lst in (("wq", wq, WQ), ("wk", wk, WK),
                                   ("wv", wv, WV)):
                for dc in range(4):
                    w = wpool.tile([P, D], F32R, tag=f"{name}{dc}", name=f"{name}{dc}")
                    nc.sync.dma_start(w[:], src[P * dc:P * (dc + 1), :])
                    lst.append(w)

            # Q^T projection from pre-gathered xqT
            for sn in range(2):
                xq = []
                for dc in range(4):
                    t = xload.tile([P, 512], F32R, tag="xchunk", name="xchunk")
                    nc.sync.dma_start(
                        t[:], xqT[P * dc:P * (dc + 1), 512 * sn:512 * (sn + 1)])
                    xq.append(t)
                for ct in range(4):
                    pq = psp.tile([P, 512], F32, tag="pp")
                    for dc in range(4):
                        nc.tensor.matmul(pq[:], WQ[dc][:, P * ct:P * (ct + 1)],
                                         xq[dc][:], start=(dc == 0),
                                         stop=(dc == 3))
                    nc.vector.tensor_scalar_add(
                        QT[ct][:, 512 * sn:512 * (sn + 1)], pq[:],
                        bqkc[:, ct:ct + 1])

            # K^T and V projections from xT, per 512-seq chunk
            for sn in range(4):
                xc = []
                for dc in range(4):
                    t = xload.tile([P, 512], F32R, tag="xchunk", name="xchunk")
                    nc.sync.dma_start(
                        t[:], xT[P * dc:P * (dc + 1), 512 * sn:512 * (sn + 1)])
                    xc.append(t)
                for ct in range(4):
                    pk = psp.tile([P, 512], F32, tag="pp")
                    for dc in range(4):
                        nc.tensor.matmul(pk[:], WK[dc][:, P * ct:P * (ct + 1)],
                                         xc[dc][:], start=(dc == 0),
                                         stop=(dc == 3))
                    nc.vector.tensor_scalar_add(
                        KT[ct][:, 512 * sn:512 * (sn + 1)], pk[:],
                        bqkc[:, 4 + ct:5 + ct])
                for k in range(4):
                    st = 4 * sn + k
                    pv = psp.tile([P, 512], F32, tag="pp")
                    for dc in range(4):
                        nc.tensor.matmul(pv[:], xc[dc][:, P * k:P * (k + 1)],
                                         WV[dc][:], start=(dc == 0), stop=False)
                    nc.tensor.matmul(pv[:], ones_r[:, 0:P], bvr[:],
                                     start=False, stop=True)
                    nc.vector.tensor_copy(
                        Vv[:, st, :, 0:HD],
                        pv[:].rearrange("p (h c) -> p h c", c=HD))
            # ones columns of V (softmax-denominator trick)
            nc.vector.tensor_copy(
                Vv[:, :, :, HD],
                ones_blk[:].rearrange("p (t h) -> p t h", h=8))

        # ---- attention + output projection ----
        with (
            tc.tile_pool(name="pexp", bufs=6) as pexp,
            tc.tile_pool(name="rcp", bufs=4) as rcp,
            tc.tile_pool(name="rbp", bufs=4) as rbp,
            tc.tile_pool(name="ostage", bufs=2) as ostage,
            tc.tile_pool(name="psg", bufs=2, space="PSUM") as psg,
            tc.tile_pool(name="psa", bufs=2, space="PSUM") as psa,
            tc.tile_pool(name="pso", bufs=2, space="PSUM") as pso,
        ):
            for j in range(SLOTS):
                qsl = slice(QS * j, QS * (j + 1))
                for h in range(H):
                    ct, hh = h // 2, (h % 2) * HD
                    pes = []
                    for g in range(j + 1):
                        pg = psg.tile([P, 4 * QS], F32, tag="pg")
                        for t4 in range(4):
                            t = 4 * g + t4
                            nc.tensor.matmul(
                                pg[:, QS * t4:QS * (t4 + 1)],
                                KT[ct][hh:hh + HD, P * t:P * (t + 1)],
                                QT[ct][hh:hh + HD, qsl],
                                start=True, stop=True)
                        pe = pexp.tile([P, 4 * QS], F32R, tag="pe")
                        nc.scalar.activation(pe[:], pg[:], AF.Exp, scale=SCALE)
                        if g == j:
                            nc.vector.tensor_mul(
                                pe[:], pe[:],
                                MK[:, 4 * QS * j:4 * QS * (j + 1)])
                        pes.append(pe)
                    pav = psa.tile([HD + 1, QS], F32, tag="pav")
                    n_t = 4 * (j + 1)
                    for t in range(n_t):
                        nc.tensor.matmul(
                            pav[:], Vv[:, t, h, :],
                            pes[t // 4][:, QS * (t % 4):QS * (t % 4 + 1)],
                            start=(t == 0), stop=(t == n_t - 1))
                    rc = rcp.tile([1, QS], F32, tag="rc")
                    nc.vector.reciprocal(rc[:], pav[HD:HD + 1, :])
                    rb = rbp.tile([HD, QS], F32, tag="rb")
                    nc.gpsimd.partition_broadcast(rb[:], rc[:])
                    nc.vector.tensor_mul(
                        VN[ct][hh:hh + HD, qsl], pav[0:HD, :], rb[:])
                for qi in range(2):
                    qt = 2 * j + qi
                    po = pso.tile([P, D], F32, tag="po")
                    for dc in range(4):
                        nc.tensor.matmul(po[:], VN[dc][:, P * qt:P * (qt + 1)],
                                         WO[dc][:], start=(dc == 0), stop=False)
                    nc.tensor.matmul(po[:], ones_r[:, 0:P], bor[:],
                                     start=False, stop=True)
                    ob = ostage.tile([P, D], F32, tag="ob")
                    nc.vector.tensor_copy(ob[:], po[:])
                    nc.sync.dma_start(out[P * qt:P * (qt + 1), :], ob[:])


def _prep(x, mask, Wqkv, bqkv, Wo, bo):
    """Host-side sharding prep. Returns per-core input maps."""
    Wr = Wqkv.reshape(D, H, 3, HD)
    wq = np.ascontiguousarray(Wr[:, :, 0, :].reshape(D, D))
    wk = np.ascontiguousarray(Wr[:, :, 1, :].reshape(D, D))
    wv = np.ascontiguousarray(Wr[:, :, 2, :].reshape(D, D))
    br = bqkv.reshape(H, 3, HD)
    bq = br[:, 0, :].reshape(1, D)
    bk = br[:, 1, :].reshape(1, D)
    bv_row = np.ascontiguousarray(br[:, 2, :].reshape(1, D))
    bqk_cols = np.ascontiguousarray(np.concatenate(
        [bq.reshape(4, P).T, bk.reshape(4, P).T], axis=1)).astype(np.float32)
    bo_row = np.ascontiguousarray(bo.reshape(1, D))
    wo = np.ascontiguousarray(Wo)

    maskps = []
    for parity in range(2):
        mk = np.empty((SLOTS, 4, P, QS), np.float32)
        for j in range(SLOTS):
            qs = QS * (2 * j + parity)
            sl = mask[qs:qs + QS, 512 * j:512 * (j + 1)]       # [256 q, 512 kv]
            mT = (sl.T == 0.0).astype(np.float32)              # [512 kv, 256 q]
            mk[j] = mT.reshape(4, P, QS)
        # -> [P, slots*4*QS]
        maskps.append(np.ascontiguousarray(
            mk.transpose(2, 0, 1, 3).reshape(P, SLOTS * 4 * QS)))

    in_maps = []
    for c in range(N_CORES):
        b, parity = c // 2, c % 2
        xb = x[b]                                              # [S, D]
        xTb = np.ascontiguousarray(xb.T)                       # [D, S]
        qrows = np.concatenate(
            [np.arange(QS * (2 * j + parity), QS * (2 * j + parity) + QS)
             for j in range(SLOTS)])
        xqTb = np.ascontiguousarray(xb[qrows].T)               # [D, NQ]
        in_maps.append(dict(
            xT=xTb, xqT=xqTb, wq=wq, wk=wk, wv=wv, wo=wo,
            bqk_cols=bqk_cols, bv_row=bv_row, bo_row=bo_row,
            maskp=maskps[parity]))
    return in_maps


def _assemble(results):
    out = np.empty((B, S, D), np.float32)
    for c in range(N_CORES):
        b, parity = c // 2, c % 2
        oc = results[c]["out"]                                 # [NQ, D]
        for j in range(SLOTS):
            qs = QS * (2 * j + parity)
            out[b, qs:qs + QS, :] = oc[QS * j:QS * (j + 1), :]
    return out


def kernel(x, mask, Wqkv, bqkv, Wo, bo):
    x = np.asarray(x, np.float32)
    mask = np.asarray(mask, np.float32)
    Wqkv = np.asarray(Wqkv, np.float32)
    bqkv = np.asarray(bqkv, np.float32)
    Wo = np.asarray(Wo, np.float32)
    bo = np.asarray(bo, np.float32)

    nc = _build()
    in_maps = _prep(x, mask, Wqkv, bqkv, Wo, bo)
    from concourse.bass_utils import run_bass_kernel_spmd
    res = run_bass_kernel_spmd(nc, in_maps, list(range(N_CORES)))
    return _assemble(res.results)
